# revision 2
# baseline (speedup 1.0000x reference)
"""CMoERouter (VQ codebook router) Trainium2 kernel.

Inputs:  x (16, 4096, 1024) fp32, centroids (8, 1024) fp32
Outputs: weights     = softmax(-cdist(x, centroids), -1)  (16, 4096, 8) fp32
         assignments = argmin_k cdist(x, centroids)       (16, 4096)    int32

Data-parallel over 8 NeuronCores (8192 tokens/core); centroids replicated.

Per-core pipeline (64 tiles of 128 tokens, token->partition mapping is
p-major so every DMA moves >=2KB contiguous per partition):
- SWDGE cast-DMA loads x fp32->fp16 (HBM read is the roofline).
- TensorE transposes each [128,128] fp16 chunk via identity matmul into
  PSUM; copy-back to SBUF is split between ScalarE and VectorE.
- Dots: 8 accumulating fp16 matmuls per tile; the stationary operand is
  the transposed x chunk, the moving operand is [c_hi | c_lo] centroid
  columns (hi/lo split removes the centroid-side fp16 rounding error).
- ||x||^2 via one ScalarE Square+accumulate pass per tile (norm error is
  k-uniform per token => softmax/argmin provably insensitive).
- Tail (once): assemble squared distances, one Sqrt, one Exp, segmented
  reductions for softmax and argmin (first-index tie-break).
Host post-pass: tokens whose top-2 weight ratio implies a distance gap
< 3e-4 (~1-2% of tokens; fp16 dot error <= ~3e-5) are recomputed in
fp32 on host, making assignments exact vs the fp32 reference.
"""
import sys
import os
import numpy as np
from contextlib import ExitStack

for _p in ('/opt/trn_rl_repo', os.path.expanduser('~/.axon_site/_ro/trn_rl_repo')):
    if os.path.isdir(_p) and _p not in sys.path:
        sys.path.insert(0, _p)

import concourse.bacc as bacc
import concourse.tile as tile
from concourse import bass_utils, mybir

D = 1024
K = 8
NCHUNK = D // 128
N_CORES = 8
N_CORE = 16 * 4096 // N_CORES   # 8192 tokens per core
NT = N_CORE // 128              # 64 tiles per core
BIG = 1000.0

F16 = mybir.dt.float16
F32 = mybir.dt.float32
I32 = mybir.dt.int32
AF = mybir.ActivationFunctionType


def build_core_kernel(group=4, xf_bufs=8, xt_bufs=8, ps_bufs=4):
    nc = bacc.Bacc("TRN2", target_bir_lowering=False, debug=False)
    x_d = nc.dram_tensor("x", [N_CORE, D], F32, kind="ExternalInput").ap()
    ct_d = nc.dram_tensor("ct", [128, NCHUNK * 16], F16, kind="ExternalInput").ap()
    cn_d = nc.dram_tensor("cn", [128, K], F32, kind="ExternalInput").ap()
    io_d = nc.dram_tensor("io", [128, K], F32, kind="ExternalInput").ap()
    id_d = nc.dram_tensor("ident", [128, 128], F16, kind="ExternalInput").ap()
    w_d = nc.dram_tensor("w", [N_CORE, K], F32, kind="ExternalOutput").ap()
    a_d = nc.dram_tensor("a", [N_CORE], I32, kind="ExternalOutput").ap()

    xv = x_d.rearrange("(p i) d -> p i d", p=128)
    wv = w_d.rearrange("(p i) k -> p i k", p=128)
    av = a_d.rearrange("(p i) -> p i", p=128)

    with tile.TileContext(nc) as tc, ExitStack() as ctx:
        cpool = ctx.enter_context(tc.tile_pool(name="const", bufs=1))
        xpool = ctx.enter_context(tc.tile_pool(name="xf", bufs=xf_bufs))
        tpool = ctx.enter_context(tc.tile_pool(name="xt", bufs=xt_bufs))
        spool = ctx.enter_context(tc.tile_pool(name="scr", bufs=2))
        bpool = ctx.enter_context(tc.tile_pool(name="big", bufs=1))
        psum = ctx.enter_context(tc.tile_pool(name="ps", bufs=ps_bufs, space="PSUM"))
        tppool = ctx.enter_context(tc.tile_pool(name="tps", bufs=3, space="PSUM"))

        ct = cpool.tile([128, NCHUNK * 16], F16)
        nc.gpsimd.dma_start(ct[:], ct_d)
        cn = cpool.tile([128, K], F32)
        nc.gpsimd.dma_start(cn[:], cn_d)
        io = cpool.tile([128, K], F32)
        nc.gpsimd.dma_start(io[:], io_d)
        ident = cpool.tile([128, 128], F16)
        nc.gpsimd.dma_start(ident[:], id_d)

        dabuf = bpool.tile([128, NT * 16], F32)
        normsb = bpool.tile([128, NT], F32)

        def load_one(i):
            xf16 = xpool.tile([128, D], F16, tag="xf16")
            nc.gpsimd.dma_start(xf16[:], xv[:, i])
            return xf16

        def trans_one(xf16):
            # PE transpose (identity matmul) to PSUM, copy back split ACT/DVE
            xt = tpool.tile([128, D], F16, tag="xt")
            xt_ps = tppool.tile([128, D], F16, tag="xtps")
            for ci in range(NCHUNK):
                nc.tensor.transpose(xt_ps[:, ci * 128:(ci + 1) * 128],
                                    xf16[:, ci * 128:(ci + 1) * 128], ident[:])
            nc.scalar.activation(xt[:, :512], xt_ps[:, :512], AF.Copy)
            nc.vector.tensor_copy(xt[:, 512:], xt_ps[:, 512:])
            return xt

        def compute_one(i, xf16, xt):
            scratch = spool.tile([128, D], F16, tag="scr")
            nc.scalar.activation(scratch[:], xf16[:], AF.Square,
                                 accum_out=normsb[:, i:i + 1])
            dots_ps = psum.tile([128, 16], F32, tag="dots")
            for ci in range(NCHUNK):
                nc.tensor.matmul(
                    dots_ps[:],
                    lhsT=xt[:, ci * 128:(ci + 1) * 128],
                    rhs=ct[:, ci * 16:(ci + 1) * 16],
                    start=(ci == 0),
                    stop=(ci == NCHUNK - 1),
                )
            nc.vector.tensor_scalar(
                out=dabuf[:, i * 16:(i + 1) * 16],
                in0=dots_ps[:],
                scalar1=-2.0,
                scalar2=None,
                op0=mybir.AluOpType.mult,
            )

        for g0 in range(0, NT, group):
            g = list(range(g0, min(g0 + group, NT)))
            xfs = {i: load_one(i) for i in g}
            xts = {i: trans_one(xfs[i]) for i in g}
            for i in g:
                compute_one(i, xfs[i], xts[i])

        # ---- tail: distances, softmax, argmin ----
        da3 = dabuf[:].rearrange("p (i h k) -> p i h k", h=2, k=K)
        sq = bpool.tile([128, NT, K], F32)
        nc.vector.tensor_tensor(out=sq[:], in0=da3[:, :, 0], in1=da3[:, :, 1],
                                op=mybir.AluOpType.add)
        nb = normsb[:].unsqueeze(2).broadcast_to([128, NT, K])
        nc.vector.tensor_tensor(out=sq[:], in0=sq[:], in1=nb,
                                op=mybir.AluOpType.add)
        cnb = cn[:].unsqueeze(1).broadcast_to([128, NT, K])
        nc.vector.tensor_tensor(out=sq[:], in0=sq[:], in1=cnb,
                                op=mybir.AluOpType.add)

        dists = bpool.tile([128, NT, K], F32)
        nc.scalar.activation(dists[:], sq[:], AF.Sqrt)

        mn = bpool.tile([128, NT], F32)
        nc.vector.tensor_reduce(mn[:].unsqueeze(2), dists[:],
                                axis=mybir.AxisListType.X, op=mybir.AluOpType.min)

        expd = bpool.tile([128, NT, K], F32)
        nc.scalar.activation(expd[:], dists[:], AF.Exp, scale=-1.0)

        ssum = bpool.tile([128, NT], F32)
        nc.vector.tensor_reduce(ssum[:].unsqueeze(2), expd[:],
                                axis=mybir.AxisListType.X, op=mybir.AluOpType.add)
        rec = bpool.tile([128, NT], F32)
        nc.vector.reciprocal(rec[:], ssum[:])

        wout = bpool.tile([128, NT, K], F32)
        rb = rec[:].unsqueeze(2).broadcast_to([128, NT, K])
        nc.vector.tensor_tensor(out=wout[:], in0=expd[:], in1=rb,
                                op=mybir.AluOpType.mult)
        nc.sync.dma_start(wv, wout[:])

        eq = bpool.tile([128, NT, K], F32)
        mb = mn[:].unsqueeze(2).broadcast_to([128, NT, K])
        nc.vector.tensor_tensor(out=eq[:], in0=dists[:], in1=mb,
                                op=mybir.AluOpType.is_equal)
        t1 = bpool.tile([128, NT, K], F32)
        nc.vector.tensor_scalar(out=t1[:], in0=eq[:], scalar1=-BIG, scalar2=None,
                                op0=mybir.AluOpType.mult)
        iob = io[:].unsqueeze(1).broadcast_to([128, NT, K])
        nc.vector.tensor_tensor(out=t1[:], in0=t1[:], in1=iob,
                                op=mybir.AluOpType.add)
        idxf = bpool.tile([128, NT], F32)
        nc.vector.tensor_reduce(idxf[:].unsqueeze(2), t1[:],
                                axis=mybir.AxisListType.X, op=mybir.AluOpType.min)
        idxi = bpool.tile([128, NT], I32)
        nc.vector.tensor_copy(idxi[:], idxf[:])
        nc.sync.dma_start(av, idxi[:])

    nc.compile()
    return nc


def _host_prep(centroids):
    c = centroids.astype(np.float32)
    c_hi = c.astype(np.float16)
    c_lo = (c.astype(np.float64) - c_hi.astype(np.float64)).astype(np.float16)
    ct = np.zeros((128, NCHUNK * 16), dtype=np.float16)
    for ci in range(NCHUNK):
        ct[:, ci * 16:ci * 16 + 8] = c_hi[:, ci * 128:(ci + 1) * 128].T
        ct[:, ci * 16 + 8:ci * 16 + 16] = c_lo[:, ci * 128:(ci + 1) * 128].T
    cnorm = (c.astype(np.float64) ** 2).sum(1).astype(np.float32)
    cn = np.broadcast_to(cnorm[None, :], (128, K)).copy()
    io = np.broadcast_to((np.arange(K) + BIG).astype(np.float32)[None, :],
                         (128, K)).copy()
    ident = np.eye(128, dtype=np.float16)
    return ct, cn, io, ident


_NC_CACHE = {}


def kernel(x, centroids):
    x = np.ascontiguousarray(np.asarray(x, dtype=np.float32))
    centroids = np.ascontiguousarray(np.asarray(centroids, dtype=np.float32))
    B, T, _ = x.shape
    rows_per_core = B * T // N_CORES
    ct, cn, io, ident = _host_prep(centroids)

    if 'nc' not in _NC_CACHE:
        _NC_CACHE['nc'] = build_core_kernel()
    nc = _NC_CACHE['nc']

    xf = x.reshape(-1, D)
    in_maps = []
    for c_id in range(N_CORES):
        xc = xf[c_id * rows_per_core:(c_id + 1) * rows_per_core]
        in_maps.append({"x": np.ascontiguousarray(xc), "ct": ct, "cn": cn,
                        "io": io, "ident": ident})

    res = bass_utils.run_bass_kernel_spmd(nc, in_maps, core_ids=list(range(N_CORES)))
    w = np.concatenate([r["w"] for r in res.results], axis=0)
    a = np.concatenate([r["a"] for r in res.results], axis=0)

    # Host near-tie refinement (exact fp32 for tokens with tiny top-2 gap).
    ws = np.sort(w, axis=1)
    sus = ws[:, -2] > ws[:, -1] * np.float32(np.exp(-3e-4))
    idx = np.nonzero(sus)[0]
    if idx.size:
        xs = xf[idx]
        sqs = ((xs * xs).sum(1, keepdims=True)
               + (centroids * centroids).sum(1)[None, :]
               - 2.0 * (xs @ centroids.T))
        dd = np.sqrt(np.maximum(sqs, 0.0))
        e = np.exp(-dd + dd.min(axis=1, keepdims=True))
        w[idx] = (e / e.sum(axis=1, keepdims=True)).astype(np.float32)
        a[idx] = dd.argmin(axis=1).astype(np.int32)

    return w.reshape(B, T, K), a.reshape(B, T).astype(np.int32)


# revision 3
# speedup vs baseline: 2.3834x; 2.3834x over previous
"""CMoERouter (VQ codebook router) Trainium2 kernel.

Inputs:  x (16, 4096, 1024) fp32, centroids (8, 1024) fp32
Outputs: weights     = softmax(-cdist(x, centroids), -1)  (16, 4096, 8) fp32
         assignments = argmin_k cdist(x, centroids)       (16, 4096)    int32

Data-parallel over 8 NeuronCores (8192 tokens/core); centroids replicated.

Per-core pipeline (64 tiles of 128 tokens, token->partition mapping is
p-major so every DMA moves >=2KB contiguous per partition):
- SWDGE cast-DMA loads x fp32->fp16 (HBM read is the roofline).
- TensorE transposes each [128,128] fp16 chunk via identity matmul into
  PSUM; copy-back to SBUF is split between ScalarE and VectorE.
- Dots: 8 accumulating fp16 matmuls per tile; the stationary operand is
  the transposed x chunk, the moving operand is [c_hi | c_lo] centroid
  columns (hi/lo split removes the centroid-side fp16 rounding error).
- ||x||^2 via one ScalarE Square+accumulate pass per tile (norm error is
  k-uniform per token => softmax/argmin provably insensitive).
- Tail (once): assemble squared distances, one Sqrt, one Exp, segmented
  reductions for softmax and argmin (first-index tie-break).
Host post-pass: tokens whose top-2 weight ratio implies a distance gap
< 3e-4 (~1-2% of tokens; fp16 dot error <= ~3e-5) are recomputed in
fp32 on host, making assignments exact vs the fp32 reference.
"""
import sys
import os
import numpy as np
from contextlib import ExitStack

for _p in ('/opt/trn_rl_repo', os.path.expanduser('~/.axon_site/_ro/trn_rl_repo')):
    if os.path.isdir(_p) and _p not in sys.path:
        sys.path.insert(0, _p)

import concourse.bacc as bacc
import concourse.tile as tile
from concourse import bass_utils, mybir

D = 1024
K = 8
NCHUNK = D // 128
N_CORES = 8
N_CORE = 16 * 4096 // N_CORES   # 8192 tokens per core
NT = N_CORE // 128              # 64 tiles per core
BIG = 1000.0

F16 = mybir.dt.float16
F32 = mybir.dt.float32
I32 = mybir.dt.int32
AF = mybir.ActivationFunctionType


def build_core_kernel(group=8, xf_bufs=12, xt_bufs=12, ps_bufs=4):
    nc = bacc.Bacc("TRN2", target_bir_lowering=False, debug=False)
    x_d = nc.dram_tensor("x", [N_CORE, D], F32, kind="ExternalInput").ap()
    ct_d = nc.dram_tensor("ct", [128, NCHUNK * 16], F16, kind="ExternalInput").ap()
    cn_d = nc.dram_tensor("cn", [128, K], F32, kind="ExternalInput").ap()
    io_d = nc.dram_tensor("io", [128, K], F32, kind="ExternalInput").ap()
    id_d = nc.dram_tensor("ident", [128, 128], F16, kind="ExternalInput").ap()
    w_d = nc.dram_tensor("w", [N_CORE, K], F32, kind="ExternalOutput").ap()
    a_d = nc.dram_tensor("a", [N_CORE], I32, kind="ExternalOutput").ap()

    xv = x_d.rearrange("(p i) d -> p i d", p=128)
    wv = w_d.rearrange("(p i) k -> p i k", p=128)
    av = a_d.rearrange("(p i) -> p i", p=128)

    with tile.TileContext(nc) as tc, ExitStack() as ctx:
        cpool = ctx.enter_context(tc.tile_pool(name="const", bufs=1))
        xpool = ctx.enter_context(tc.tile_pool(name="xf", bufs=xf_bufs))
        tpool = ctx.enter_context(tc.tile_pool(name="xt", bufs=xt_bufs))
        spool = ctx.enter_context(tc.tile_pool(name="scr", bufs=2))
        bpool = ctx.enter_context(tc.tile_pool(name="big", bufs=1))
        psum = ctx.enter_context(tc.tile_pool(name="ps", bufs=ps_bufs, space="PSUM"))
        tppool = ctx.enter_context(tc.tile_pool(name="tps", bufs=3, space="PSUM"))

        ct = cpool.tile([128, NCHUNK * 16], F16)
        nc.gpsimd.dma_start(ct[:], ct_d)
        cn = cpool.tile([128, K], F32)
        nc.gpsimd.dma_start(cn[:], cn_d)
        io = cpool.tile([128, K], F32)
        nc.gpsimd.dma_start(io[:], io_d)
        ident = cpool.tile([128, 128], F16)
        nc.gpsimd.dma_start(ident[:], id_d)

        dabuf = bpool.tile([128, NT * 16], F32)
        normsb = bpool.tile([128, NT], F32)

        def load_one(i):
            xf16 = xpool.tile([128, D], F16, tag="xf16")
            nc.gpsimd.dma_start(xf16[:], xv[:, i])
            return xf16

        def trans_one(xf16):
            # PE transpose (identity matmul) to PSUM, copy back split ACT/DVE
            xt = tpool.tile([128, D], F16, tag="xt")
            xt_ps = tppool.tile([128, D], F16, tag="xtps")
            for ci in range(NCHUNK):
                nc.tensor.transpose(xt_ps[:, ci * 128:(ci + 1) * 128],
                                    xf16[:, ci * 128:(ci + 1) * 128], ident[:])
            nc.scalar.activation(xt[:, :384], xt_ps[:, :384], AF.Copy)
            nc.vector.tensor_copy(xt[:, 384:], xt_ps[:, 384:])
            return xt

        def compute_one(i, xf16, xt):
            scratch = spool.tile([128, D], F16, tag="scr")
            nc.scalar.activation(scratch[:], xf16[:], AF.Square,
                                 accum_out=normsb[:, i:i + 1])
            dots_ps = psum.tile([128, 16], F32, tag="dots")
            for ci in range(NCHUNK):
                nc.tensor.matmul(
                    dots_ps[:],
                    lhsT=xt[:, ci * 128:(ci + 1) * 128],
                    rhs=ct[:, ci * 16:(ci + 1) * 16],
                    start=(ci == 0),
                    stop=(ci == NCHUNK - 1),
                )
            nc.vector.tensor_scalar(
                out=dabuf[:, i * 16:(i + 1) * 16],
                in0=dots_ps[:],
                scalar1=-2.0,
                scalar2=None,
                op0=mybir.AluOpType.mult,
            )

        for g0 in range(0, NT, group):
            g = list(range(g0, min(g0 + group, NT)))
            xfs = {i: load_one(i) for i in g}
            xts = {i: trans_one(xfs[i]) for i in g}
            for i in g:
                compute_one(i, xfs[i], xts[i])

        # ---- tail: distances, softmax, argmin ----
        da3 = dabuf[:].rearrange("p (i h k) -> p i h k", h=2, k=K)
        sq = bpool.tile([128, NT, K], F32)
        nc.vector.tensor_tensor(out=sq[:], in0=da3[:, :, 0], in1=da3[:, :, 1],
                                op=mybir.AluOpType.add)
        nb = normsb[:].unsqueeze(2).broadcast_to([128, NT, K])
        nc.vector.tensor_tensor(out=sq[:], in0=sq[:], in1=nb,
                                op=mybir.AluOpType.add)
        cnb = cn[:].unsqueeze(1).broadcast_to([128, NT, K])
        nc.vector.tensor_tensor(out=sq[:], in0=sq[:], in1=cnb,
                                op=mybir.AluOpType.add)

        dists = bpool.tile([128, NT, K], F32)
        nc.scalar.activation(dists[:], sq[:], AF.Sqrt)

        mn = bpool.tile([128, NT], F32)
        nc.vector.tensor_reduce(mn[:].unsqueeze(2), dists[:],
                                axis=mybir.AxisListType.X, op=mybir.AluOpType.min)

        expd = bpool.tile([128, NT, K], F32)
        nc.scalar.activation(expd[:], dists[:], AF.Exp, scale=-1.0)

        ssum = bpool.tile([128, NT], F32)
        nc.vector.tensor_reduce(ssum[:].unsqueeze(2), expd[:],
                                axis=mybir.AxisListType.X, op=mybir.AluOpType.add)
        rec = bpool.tile([128, NT], F32)
        nc.vector.reciprocal(rec[:], ssum[:])

        wout = bpool.tile([128, NT, K], F32)
        rb = rec[:].unsqueeze(2).broadcast_to([128, NT, K])
        nc.vector.tensor_tensor(out=wout[:], in0=expd[:], in1=rb,
                                op=mybir.AluOpType.mult)
        nc.sync.dma_start(wv, wout[:])

        eq = bpool.tile([128, NT, K], F32)
        mb = mn[:].unsqueeze(2).broadcast_to([128, NT, K])
        nc.vector.tensor_tensor(out=eq[:], in0=dists[:], in1=mb,
                                op=mybir.AluOpType.is_equal)
        t1 = bpool.tile([128, NT, K], F32)
        nc.vector.tensor_scalar(out=t1[:], in0=eq[:], scalar1=-BIG, scalar2=None,
                                op0=mybir.AluOpType.mult)
        iob = io[:].unsqueeze(1).broadcast_to([128, NT, K])
        nc.vector.tensor_tensor(out=t1[:], in0=t1[:], in1=iob,
                                op=mybir.AluOpType.add)
        idxf = bpool.tile([128, NT], F32)
        nc.vector.tensor_reduce(idxf[:].unsqueeze(2), t1[:],
                                axis=mybir.AxisListType.X, op=mybir.AluOpType.min)
        idxi = bpool.tile([128, NT], I32)
        nc.vector.tensor_copy(idxi[:], idxf[:])
        nc.sync.dma_start(av, idxi[:])

    nc.compile()
    return nc


def _host_prep(centroids):
    c = centroids.astype(np.float32)
    c_hi = c.astype(np.float16)
    c_lo = (c.astype(np.float64) - c_hi.astype(np.float64)).astype(np.float16)
    ct = np.zeros((128, NCHUNK * 16), dtype=np.float16)
    for ci in range(NCHUNK):
        ct[:, ci * 16:ci * 16 + 8] = c_hi[:, ci * 128:(ci + 1) * 128].T
        ct[:, ci * 16 + 8:ci * 16 + 16] = c_lo[:, ci * 128:(ci + 1) * 128].T
    cnorm = (c.astype(np.float64) ** 2).sum(1).astype(np.float32)
    cn = np.broadcast_to(cnorm[None, :], (128, K)).copy()
    io = np.broadcast_to((np.arange(K) + BIG).astype(np.float32)[None, :],
                         (128, K)).copy()
    ident = np.eye(128, dtype=np.float16)
    return ct, cn, io, ident


_NC_CACHE = {}


def kernel(x, centroids):
    x = np.ascontiguousarray(np.asarray(x, dtype=np.float32))
    centroids = np.ascontiguousarray(np.asarray(centroids, dtype=np.float32))
    B, T, _ = x.shape
    rows_per_core = B * T // N_CORES
    ct, cn, io, ident = _host_prep(centroids)

    if 'nc' not in _NC_CACHE:
        _NC_CACHE['nc'] = build_core_kernel()
    nc = _NC_CACHE['nc']

    xf = x.reshape(-1, D)
    in_maps = []
    for c_id in range(N_CORES):
        xc = xf[c_id * rows_per_core:(c_id + 1) * rows_per_core]
        in_maps.append({"x": np.ascontiguousarray(xc), "ct": ct, "cn": cn,
                        "io": io, "ident": ident})

    res = bass_utils.run_bass_kernel_spmd(nc, in_maps, core_ids=list(range(N_CORES)))
    w = np.concatenate([r["w"] for r in res.results], axis=0)
    a = np.concatenate([r["a"] for r in res.results], axis=0)

    # Host near-tie refinement (exact fp32 for tokens with tiny top-2 gap).
    ws = np.sort(w, axis=1)
    sus = ws[:, -2] > ws[:, -1] * np.float32(np.exp(-3e-4))
    idx = np.nonzero(sus)[0]
    if idx.size:
        xs = xf[idx]
        sqs = ((xs * xs).sum(1, keepdims=True)
               + (centroids * centroids).sum(1)[None, :]
               - 2.0 * (xs @ centroids.T))
        dd = np.sqrt(np.maximum(sqs, 0.0))
        e = np.exp(-dd + dd.min(axis=1, keepdims=True))
        w[idx] = (e / e.sum(axis=1, keepdims=True)).astype(np.float32)
        a[idx] = dd.argmin(axis=1).astype(np.int32)

    return w.reshape(B, T, K), a.reshape(B, T).astype(np.int32)
